# revision 37
# baseline (speedup 1.0000x reference)
"""Trainium2 Bass kernel for nn_Attention_85813446574600.

Reference computes:
    s_x = x @ W[:F] + b            # [B,T,1]
    s_c = context @ W[F:]          # [C,1]
    scores = s_x + s_c             # [B,T,C,1]
    att = softmax(scores, axis=-1) # softmax over a SIZE-1 axis -> exactly 1.0
    out = einsum('btc,btf->bcf', att, x)

Since softmax over the last (size-1) axis is identically 1.0 for any finite
scores, the output is exactly out[b,c,f] = sum_t x[b,t,f], independent of c
(and of context/W/b entirely).

Device kernel (per core, batch-sharded 32/8 = 4 batches), raw Bass:

  sync engine   : input DMAs on the qSP HWDGE ring: one [128, 4*F] tile per
                  batch for b0..b2 (8KB contiguous per partition), and two
                  [128, 2*F] half tiles for b3 so the tail dependency is a
                  512KB load. This descriptor shape measurably keeps the 16
                  DMA engines in lockstep (~0.3us chunk-completion skew vs
                  ~3us with 8 uniform 512KB descriptors). Output rows go out
                  as tiny per-batch [1,F] DMAs on the same ring.
  vector engine : per batch two adds folding the 4 T-rows per partition into
                  one: [.,2F]+[.,2F] in fp32, then [.,F]+[.,F] with bf16
                  output (the bf16 downconvert rides the add for free). b3's
                  halves each get a single fp32->bf16 add; the cross-half
                  sum happens in PSUM via matmul accumulation.
  tensor engine : per batch, accumulating bf16 matmuls with an all-ones
                  [128,4] stationary reduce the 128 partitions into PSUM
                  (fp32) at 1 cycle/row. Warm-up/filler matmuls keep the PE
                  pstate at the warm clock for the tail-critical matmuls.
  scalar engine : a dummy activation up front to preload the ACT table off
                  the critical path, then per batch one [1,F] PSUM->SBUF
                  copy feeding the output DMA.

The device writes only the [B_LOC, F] row sums (8KB); replicating them over
the C axis (exact, by construction of the math above) happens while
unsharding on the host. The 4MB/core input stream at the ~435 GB/s per-core
DMA cap is the floor; everything else is pipelined behind it.
"""

import sys

for _p in ("/opt/trn_rl_repo",):
    if _p not in sys.path:
        sys.path.insert(0, _p)

from contextlib import ExitStack

import numpy as np

import concourse.bass as bass
import concourse.mybir as mybir
from concourse.bass_utils import run_bass_kernel_spmd

# Problem shapes (hardcoded per harness contract)
B, T, C, F = 32, 512, 256, 512
N_CORES = 8
B_LOC = B // N_CORES  # 4 batches per core
P = 128               # SBUF/PSUM partitions
TT = T // P           # 4 T-rows folded into each partition (b0..b2 tiles)
DT = mybir.dt.float32
BF = mybir.dt.bfloat16

_NC_CACHE = {}


def _build_nc():
    # Bass.__init__ ends with const-AP memsets plus an all-engine barrier;
    # nothing in this kernel reads the const APs and every cross-engine
    # dependency is explicitly semaphore-gated, so skip that barrier to
    # issue the first input DMA sooner.
    _orig_barrier = bass.Bass.all_engine_barrier
    bass.Bass.all_engine_barrier = lambda self, sem_only=False: None
    try:
        nc = bass.Bass("TRN2", target_bir_lowering=False)
    finally:
        bass.Bass.all_engine_barrier = _orig_barrier
    x = nc.dram_tensor("x", [B_LOC, T, F], DT, kind="ExternalInput").ap()
    out = nc.dram_tensor("out", [B_LOC, F], DT, kind="ExternalOutput").ap()

    with ExitStack() as ctx:
        ec = ctx.enter_context
        ones = ec(nc.sbuf_tensor("ones", [P, P], BF)).ap()
        # b0..b2: one [128, 4*F] tile each (8KB/partition); b3: two halves
        xts = [ec(nc.sbuf_tensor(f"xt{b}", [P, TT * F], DT)).ap() for b in range(3)]
        xt3a = ec(nc.sbuf_tensor("xt3a", [P, 2 * F], DT)).ap()
        xt3b = ec(nc.sbuf_tensor("xt3b", [P, 2 * F], DT)).ap()
        # bf16 half sums: two per batch (u = l0+l1, v = l2+l3)
        uvs = [
            [
                ec(nc.sbuf_tensor(f"uv{b}{h}", [P, F], BF)).ap()
                for h in range(2)
            ]
            for b in range(3)
        ]
        t3a = ec(nc.sbuf_tensor("t3a", [P, F], BF)).ap()
        t3b = ec(nc.sbuf_tensor("t3b", [P, F], BF)).ap()
        osbs = [
            ec(nc.sbuf_tensor(f"osb{b}", [1, F], DT)).ap() for b in range(B_LOC)
        ]
        scratch = ec(nc.sbuf_tensor("scratch", [1, 16], DT)).ap()
        psums = [
            ec(nc.psum_tensor(f"acc{b}", [4, F], DT)).ap() for b in range(B_LOC)
        ]
        warm_ps = ec(nc.psum_tensor("warm_ps", [P, P], DT)).ap()

        in_sems = [ec(nc.semaphore(f"in_sem{b}")) for b in range(3)]
        in3a_sem = ec(nc.semaphore("in3a_sem"))
        in3b_sem = ec(nc.semaphore("in3b_sem"))
        ones_sem = ec(nc.semaphore("ones_sem"))
        add_sem = ec(nc.semaphore("add_sem"))
        pe_sem = ec(nc.semaphore("pe_sem"))
        peL_sem = ec(nc.semaphore("peL_sem"))
        peR_sem = ec(nc.semaphore("peR_sem"))
        cp_sem = ec(nc.semaphore("cp_sem"))
        cpL_sem = ec(nc.semaphore("cpL_sem"))
        cpR_sem = ec(nc.semaphore("cpR_sem"))
        osem = ec(nc.semaphore("osem"))

        block = ec(nc.Block(no_gpsimd_drain=True))

        @block.sync
        def _(sync):
            # b3's first half leads the stream so only its tiny second half
            # is on the tail; the last descriptor's compute chain is minimal.
            # b3 halves: partition p of half h <- x[3, h*256+2p : +2, :]
            src3 = x[3].rearrange("(h p l) f -> h p l f", h=2, p=P)
            sync.dma_start(
                xt3a.rearrange("p (l f) -> p l f", l=2), src3[0]
            ).then_inc(in3a_sem, 16)
            # batch tiles b0..b2: partition p <- x[b, 4p:4p+4, :]
            for b in range(3):
                src = x[b].rearrange("(p l) f -> p l f", p=P)
                sync.dma_start(
                    xts[b].rearrange("p (l f) -> p l f", l=TT), src
                ).then_inc(in_sems[b], 16)
            sync.dma_start(
                xt3b.rearrange("p (l f) -> p l f", l=2), src3[1]
            ).then_inc(in3b_sem, 16)
            # per-batch [1,F] output rows on the same (hot) ring
            for b in range(3):
                sync.wait_ge(cp_sem, b + 1)
                sync.dma_start(out[b : b + 1, :], osbs[b]).then_inc(osem, 16)
            sync.wait_ge(cpR_sem, 1)
            sync.dma_start(out[3:4, :], osbs[3]).then_inc(osem, 16)
            sync.wait_ge(osem, 16 * B_LOC)

        @block.vector
        def _(vector):
            nc.vector.memset(ones, 1.0).then_inc(ones_sem, 1)
            vector.wait_ge(in3a_sem, 16)
            nc.vector.tensor_add(t3a, xt3a[:, 0:F], xt3a[:, F : 2 * F]).then_inc(
                add_sem, 1
            )
            # per batch: two independent adds (no same-engine RAW chain),
            # each folding 2 T-rows with the bf16 downconvert riding along
            for b in range(3):
                vector.wait_ge(in_sems[b], 16)
                nc.vector.tensor_add(
                    uvs[b][0], xts[b][:, 0:F], xts[b][:, F : 2 * F]
                ).then_inc(add_sem, 1)
                nc.vector.tensor_add(
                    uvs[b][1], xts[b][:, 2 * F : 3 * F], xts[b][:, 3 * F : 4 * F]
                ).then_inc(add_sem, 1)
            Fh = F // 2
            vector.wait_ge(in3b_sem, 16)
            nc.vector.tensor_add(t3b, xt3b[:, 0:F], xt3b[:, F : 2 * F]).then_inc(
                add_sem, 1
            )

        @block.tensor
        def _(tensor):
            # warm-up so the PE pstate ramps before the tail-critical matmuls
            tensor.wait_ge(ones_sem, 1)
            for _ in range(12):
                nc.tensor.matmul(warm_ps, ones, ones, start=True, stop=True)
            ones4 = ones[:, 0:4]
            tensor.wait_ge(add_sem, 1)
            nc.tensor.matmul(psums[3], ones4, t3a, start=True, stop=False)
            for b in range(3):
                tensor.wait_ge(add_sem, 2 * b + 2)
                nc.tensor.matmul(
                    psums[b], ones4, uvs[b][0], start=True, stop=False
                )
                tensor.wait_ge(add_sem, 2 * b + 3)
                nc.tensor.matmul(
                    psums[b], ones4, uvs[b][1], start=False, stop=True
                ).then_inc(pe_sem, 1)
                if b < 2:
                    for _ in range(2):
                        nc.tensor.matmul(
                            warm_ps, ones, ones, start=True, stop=True
                        )
            tensor.wait_ge(add_sem, 8)
            nc.tensor.matmul(psums[3], ones4, t3b, start=False, stop=True).then_inc(
                peL_sem, 1
            )

        @block.scalar
        def _(scalar):
            # dummy activation preloads the ACT table off the critical path
            nc.scalar.copy(scratch, scratch)
            # psum[b] rows 0..3 all hold the batch total; copy one [1,F] row
            for b in range(3):
                scalar.wait_ge(pe_sem, b + 1)
                nc.scalar.copy(osbs[b], psums[b][0:1, :]).then_inc(cp_sem, 1)
            scalar.wait_ge(peL_sem, 1)
            nc.scalar.copy(osbs[3], psums[3][0:1, :]).then_inc(cpR_sem, 1)

    return nc


def _get_nc():
    if "nc" not in _NC_CACHE:
        _NC_CACHE["nc"] = _build_nc()
    return _NC_CACHE["nc"]


def make_in_maps(x):
    return [{"x": x[i * B_LOC : (i + 1) * B_LOC]} for i in range(N_CORES)]


def kernel(x, context=None, W=None, b=None, **_unused):
    """Full inputs in, full output out. context/W/b provably do not affect
    the output (softmax over a size-1 axis is identically 1)."""
    x = np.ascontiguousarray(np.asarray(x), dtype=np.float32)
    assert x.shape == (B, T, F), x.shape

    nc = _get_nc()
    res = run_bass_kernel_spmd(
        nc, make_in_maps(x), core_ids=list(range(N_CORES))
    )
    # unshard: each core returns its [B_LOC, F] row sums; out[b,c,f] is
    # independent of c (softmax over a size-1 axis == 1), so expanding over
    # the C axis is exact replication.
    sums = np.concatenate([np.asarray(r["out"]) for r in res.results], axis=0)
    return np.ascontiguousarray(np.broadcast_to(sums[:, None, :], (B, C, F)))
